# revision 3
# baseline (speedup 1.0000x reference)
"""GraphSAGE supervised forward on 8 Trainium2 NeuronCores.

Full inputs in, full output out. Data-parallel over the B=1024 seed nodes:
128 seeds per core; neighbor rows shard as contiguous row ranges. Tiny
weights replicated.

v2 design — quantize + transpose on host, group-mean on the PE:
  - hop-2 neighbors (the 82MB/core f32 stream) are sent as fp8e4m3 in
    feat-major layout [128, 80000]; hop-1 neighbors as bf16 [128, 3200];
    seeds f32. End-to-end max rel err ~2e-3 (gate 2e-2): the two
    mean-over-25 stages attenuate per-element quantization noise.
  - group-sum of 25 neighbor columns runs on the PE as accumulating
    identity matmuls: stationary [I;I] fp8 pairs + DoubleRow perf mode
    sums 2 phase-columns per 0.5 cycles into f32 PSUM (12 DR matmuls +
    1 plain fp8 matmul per 400-group tile). No DVE tree, no transposes.
  - hT chunk = wtop^T@negT + wbot^T@redT (bf16, 1cy/row) straight into
    PSUM; hop-1 group-sum = DVE strided reduce directly from PSUM into
    n1 (hT itself is never materialized).
  - hop-1 + 4-layer MLP + softmax in f32, in 3 parts overlapped with the
    tail of the d-side stream (as in v1).
Roofline: DMA ~59us (20.5MB fp8 + 1.6MB bf16 @ 360GB/s), PE ~25us,
DVE ~10us, ACT ~8us -> DMA-bound.
"""

import sys

for _p in ("/opt/trn_rl_repo", "/root/.axon_site/_ro/trn_rl_repo"):
    if _p not in sys.path:
        sys.path.append(_p)

import numpy as np
import ml_dtypes
from contextlib import ExitStack

import concourse.bass as bass
import concourse.tile as tile
from concourse import bacc, mybir
from concourse.bass_utils import run_bass_kernel_spmd

B, S, D = 1024, 25, 128
NCORES = 8
BL = B // NCORES          # 128 seeds per core
G1 = BL * S               # 3200 hop-1 rows (= hop-2 groups) per core
G2 = G1 * S               # 80000 hop-2 rows per core

TG = 400                  # groups per stream tile
TC = TG * S               # 10000 columns per stream tile
NT = G1 // TG             # 8 stream tiles per side
SPT = TG // S             # 16 seeds completed per tile

F32 = mybir.dt.float32
BF16 = mybir.dt.bfloat16
F8 = mybir.dt.float8e4
AX = mybir.AxisListType
AF = mybir.ActivationFunctionType
DR = mybir.MatmulPerfMode.DoubleRow

NPF8 = ml_dtypes.float8_e4m3
NPBF = ml_dtypes.bfloat16


def _build_program():
    nc = bacc.Bacc("TRN2", target_bir_lowering=False, debug=False)

    ins = {}
    for side in ("s", "d"):
        ins[f"seed_{side}"] = nc.dram_tensor(f"seed_{side}", [D, BL], F32, kind="ExternalInput")
        ins[f"neg_{side}"] = nc.dram_tensor(f"neg_{side}", [D, G1], BF16, kind="ExternalInput")
        ins[f"nn_{side}"] = nc.dram_tensor(f"nn_{side}", [D, G2], F8, kind="ExternalInput")
    for name, shape, dt in (
        ("ident2", [D, 2 * D], F8),
        ("wtopb", [D, D], BF16), ("wbotb", [D, D], BF16),
        ("wtop32", [D, D], F32), ("wbot32", [D, D], F32),
        ("w1t", [D, D], F32), ("w1b", [D, D], F32),
        ("w2m", [D, 64], F32), ("w3m", [64, 8], F32), ("w4m", [8, 2], F32),
    ):
        ins[name] = nc.dram_tensor(name, shape, dt, kind="ExternalInput")
    out_dram = nc.dram_tensor("out", [BL, 2], F32, kind="ExternalOutput")

    with tile.TileContext(nc) as tc, ExitStack() as ctx:
        const = ctx.enter_context(tc.tile_pool(name="const", bufs=1))
        persist = ctx.enter_context(tc.tile_pool(name="persist", bufs=1))
        stream = ctx.enter_context(tc.tile_pool(name="stream", bufs=3))
        work = ctx.enter_context(tc.tile_pool(name="work", bufs=3))
        psA = ctx.enter_context(tc.tile_pool(name="psA", bufs=2, space="PSUM"))
        psB = ctx.enter_context(tc.tile_pool(name="psB", bufs=2, space="PSUM"))
        psM = ctx.enter_context(tc.tile_pool(name="psM", bufs=2, space="PSUM"))

        def load_const(name, shape, dt):
            t = const.tile(shape, dt, tag=name)
            nc.gpsimd.dma_start(t[:], ins[name].ap())
            return t

        # order matters: first consts feed the first stream tile's matmuls
        ident2 = load_const("ident2", [D, 2 * D], F8)
        wtopb = load_const("wtopb", [D, D], BF16)
        wbotb = load_const("wbotb", [D, D], BF16)
        negT, seedT = {}, {}
        for side in ("s", "d"):
            negT[side] = const.tile([D, G1], BF16, tag=f"negT_{side}", name=f"negT_{side}")
            nc.gpsimd.dma_start(negT[side][:], ins[f"neg_{side}"].ap())
            seedT[side] = const.tile([D, BL], F32, tag=f"seedT_{side}", name=f"seedT_{side}")
            nc.gpsimd.dma_start(seedT[side][:], ins[f"seed_{side}"].ap())
        wtop32 = load_const("wtop32", [D, D], F32)
        wbot32 = load_const("wbot32", [D, D], F32)
        w1t = load_const("w1t", [D, D], F32)
        w1b = load_const("w1b", [D, D], F32)
        w2m = load_const("w2m", [D, 64], F32)
        w3m = load_const("w3m", [64, 8], F32)
        w4m = load_const("w4m", [8, 2], F32)

        idv2 = ident2.rearrange("p (j m) -> p j m", j=2)  # [128, 2, 128]
        id1 = ident2[:, 0:D]                              # [128, 128]

        redT, n1 = {}, {}
        for side in ("s", "d"):
            redT[side] = persist.tile([D, G1], BF16, tag=f"redT_{side}", name=f"redT_{side}")
            n1[side] = persist.tile([D, BL], F32, tag=f"n1_{side}", name=f"n1_{side}")

        oT = {}
        PARTS = [(0, 64), (64, 112), (112, BL)]

        def hop1_part(side, pi):
            lo, hi = PARTS[pi]
            w = hi - lo
            ps_o = psM.tile([D, w], F32, tag="ps_m")
            nc.tensor.matmul(ps_o[:], wtop32[:], seedT[side][:, lo:hi], start=True, stop=False)
            nc.tensor.matmul(ps_o[:], wbot32[:], n1[side][:, lo:hi], start=False, stop=True)
            ot = persist.tile([D, w], F32, tag=f"oT_{side}{pi}")
            nc.scalar.activation(ot[:], ps_o[:], AF.Copy)
            oT[side, pi] = ot

        def mlp_part(pi):
            lo, hi = PARTS[pi]
            w = hi - lo
            ps1 = psM.tile([D, w], F32, tag="ps_m")
            nc.tensor.matmul(ps1[:], w1t[:], oT["s", pi][:], start=True, stop=False)
            nc.tensor.matmul(ps1[:], w1b[:], oT["d", pi][:], start=False, stop=True)
            h1 = work.tile([D, w], F32, tag="h1")
            nc.scalar.activation(h1[:], ps1[:], AF.Relu)

            ps2 = psM.tile([64, w], F32, tag="ps_m")
            nc.tensor.matmul(ps2[:], w2m[:], h1[:])
            h2 = work.tile([64, w], F32, tag="h2")
            nc.scalar.activation(h2[:], ps2[:], AF.Relu)

            ps3 = psM.tile([8, w], F32, tag="ps_m")
            nc.tensor.matmul(ps3[:], w3m[:], h2[:])
            h3 = work.tile([8, w], F32, tag="h3")
            nc.scalar.activation(h3[:], ps3[:], AF.Relu)

            ps4 = psM.tile([w, 2], F32, tag="ps_m")
            nc.tensor.matmul(ps4[:], h3[:], w4m[:])
            lg = work.tile([w, 2], F32, tag="lg")
            nc.scalar.activation(lg[:], ps4[:], AF.Copy)

            nm = work.tile([w, 1], F32, tag="nm")
            nc.vector.reduce_max(nm[:], lg[:], axis=AX.X, negate=True)
            ex = work.tile([w, 2], F32, tag="ex")
            se = work.tile([w, 1], F32, tag="se")
            nc.scalar.activation(ex[:], lg[:], AF.Exp, bias=nm[:], accum_out=se[:])
            rc = work.tile([w, 1], F32, tag="rc")
            nc.vector.reciprocal(rc[:], se[:])
            o = work.tile([w, 2], F32, tag="o")
            nc.vector.tensor_scalar_mul(o[:], ex[:], rc[:])
            # SWDGE: a sync-queue store would head-of-line block stream loads
            nc.gpsimd.dma_start(out_dram.ap()[lo:hi], o[:])

        for side in ("s", "d"):
            for t in range(NT):
                xt = stream.tile([D, TC], F8, tag="xt")
                nc.sync.dma_start(xt[:], ins[f"nn_{side}"].ap()[:, t * TC:(t + 1) * TC])
                xr = xt.rearrange("p (g s) -> p g s", s=S)
                # group-sum of 25 phase-columns on the PE: DoubleRow identity
                # matmuls accumulate phase pairs, final odd phase plain fp8
                ps = psA.tile([D, TG], F32, tag="ps_red")
                for i in range(S // 2):
                    nc.tensor.matmul(
                        ps[:], idv2,
                        xr[:, :, 2 * i:2 * i + 2].rearrange("p g s -> p s g"),
                        start=(i == 0), stop=False, perf_mode=DR,
                    )
                nc.tensor.matmul(ps[:], id1, xr[:, :, S - 1], start=False, stop=True)
                rsl = redT[side][:, t * TG:(t + 1) * TG]
                nc.scalar.activation(rsl, ps[:], AF.Copy)

                ps2 = psB.tile([D, TG], F32, tag="ps_ht")
                nc.tensor.matmul(ps2[:], wtopb[:], negT[side][:, t * TG:(t + 1) * TG], start=True, stop=False)
                nc.tensor.matmul(ps2[:], wbotb[:], rsl, start=False, stop=True)
                # hop-1 group-sum straight from PSUM; hT never hits SBUF
                nc.vector.reduce_sum(
                    n1[side][:, t * SPT:(t + 1) * SPT],
                    ps2.rearrange("p (b s) -> p b s", s=S),
                    axis=AX.X,
                )
                if side == "d":
                    if t == 3:
                        for sd in ("s", "d"):
                            hop1_part(sd, 0)
                        mlp_part(0)
                    elif t == 6:
                        for sd in ("s", "d"):
                            hop1_part(sd, 1)
                        mlp_part(1)
        for sd in ("s", "d"):
            hop1_part(sd, 2)
        mlp_part(2)

    nc.compile()
    return nc


_NC_CACHE = None


def _get_program():
    global _NC_CACHE
    if _NC_CACHE is None:
        _NC_CACHE = _build_program()
    return _NC_CACHE


def kernel(src, src_neg, src_neg_neg, dst, dst_neg, dst_neg_neg, w2, W1, W2, W3, W4,
           _trace=False, **trace_kwargs):
    nc = _get_program()

    w2 = np.asarray(w2, np.float32)
    W1 = np.asarray(W1, np.float32)
    wtop = np.ascontiguousarray(w2[:D])
    wbot = np.ascontiguousarray(w2[D:]) / np.float32(S)
    eye = np.eye(D, dtype=np.float32)
    rep = {
        "ident2": np.concatenate([eye, eye], axis=1).astype(NPF8),
        "wtopb": wtop.astype(NPBF), "wbotb": wbot.astype(NPBF),
        "wtop32": wtop, "wbot32": wbot,
        "w1t": np.ascontiguousarray(W1[:D]),
        "w1b": np.ascontiguousarray(W1[D:]),
        "w2m": np.asarray(W2, np.float32),
        "w3m": np.asarray(W3, np.float32),
        "w4m": np.asarray(W4, np.float32),
    }

    def shardT(x, dt, rows):
        # [NCORES*rows, D] -> fp8/bf16/f32, transposed per core -> [NCORES, D, rows]
        return np.ascontiguousarray(
            np.asarray(x).astype(dt).reshape(NCORES, rows, D).transpose(0, 2, 1)
        )

    big = {
        "nn_s": shardT(src_neg_neg, NPF8, G2),
        "nn_d": shardT(dst_neg_neg, NPF8, G2),
        "neg_s": shardT(src_neg, NPBF, G1),
        "neg_d": shardT(dst_neg, NPBF, G1),
        "seed_s": shardT(src, np.float32, BL),
        "seed_d": shardT(dst, np.float32, BL),
    }
    in_maps = []
    for c in range(NCORES):
        m = dict(rep)
        for k, v in big.items():
            m[k] = v[c]
        in_maps.append(m)

    res = run_bass_kernel_spmd(
        nc, in_maps, list(range(NCORES)), trace=_trace, **trace_kwargs
    )
    out = np.concatenate([res.results[c]["out"] for c in range(NCORES)], axis=0)
    if _trace:
        return out, res
    return out


# revision 6
# speedup vs baseline: 1.9224x; 1.9224x over previous
"""GraphSAGE supervised forward on 8 Trainium2 NeuronCores.

Full inputs in, full output out. Data-parallel over the B=1024 seed nodes:
128 seeds per core; neighbor rows shard as contiguous row ranges. Tiny
weights replicated.

v2 design — quantize + transpose on host, group-mean on the PE:
  - hop-2 neighbors (the 82MB/core f32 stream) are sent as fp8e4m3 in
    feat-major layout [128, 80000]; hop-1 neighbors as bf16 [128, 3200];
    seeds f32. End-to-end max rel err ~2e-3 (gate 2e-2): the two
    mean-over-25 stages attenuate per-element quantization noise.
  - group-sum of 25 neighbor columns runs on the PE as accumulating
    identity matmuls: stationary [I;I] fp8 pairs + DoubleRow perf mode
    sums 2 phase-columns per cycle pair into f32 PSUM (12 DR matmuls +
    1 plain fp8 matmul per 400-group tile). No DVE tree, no transposes.
    Host lays the stream out PHASE-MAJOR per tile (25 blocks of 400
    contiguous group-columns) so every matmul's moving operand is
    contiguous — strided 1-byte moving fetch halves PE rate.
  - hT chunk = wtop^T@negT + wbot^T@redT (bf16, 1cy/row) straight into
    PSUM; hop-1 group-sum = DVE strided reduce directly from PSUM into
    n1 (hT itself is never materialized).
  - hop-1 + 4-layer MLP + softmax in f32, in 3 parts overlapped with the
    tail of the d-side stream (as in v1).
Roofline: DMA ~59us (20.5MB fp8 + 1.6MB bf16 @ 360GB/s), PE ~25us,
DVE ~10us, ACT ~8us -> DMA-bound.
"""

import sys

for _p in ("/opt/trn_rl_repo", "/root/.axon_site/_ro/trn_rl_repo"):
    if _p not in sys.path:
        sys.path.append(_p)

import numpy as np
import ml_dtypes
from contextlib import ExitStack

import concourse.bass as bass
import concourse.tile as tile
from concourse import bacc, mybir
from concourse.bass_utils import run_bass_kernel_spmd

B, S, D = 1024, 25, 128
NCORES = 8
BL = B // NCORES          # 128 seeds per core
G1 = BL * S               # 3200 hop-1 rows (= hop-2 groups) per core
G2 = G1 * S               # 80000 hop-2 rows per core

TG = 400                  # groups per stream tile
TC = TG * S               # 10000 columns per stream tile
NT = G1 // TG             # 8 stream tiles per side
SPT = TG // S             # 16 seeds completed per tile

F32 = mybir.dt.float32
BF16 = mybir.dt.bfloat16
F8 = mybir.dt.float8e4
AX = mybir.AxisListType
AF = mybir.ActivationFunctionType
DR = mybir.MatmulPerfMode.DoubleRow

NPF8 = ml_dtypes.float8_e4m3
NPBF = ml_dtypes.bfloat16


def _build_program():
    nc = bacc.Bacc("TRN2", target_bir_lowering=False, debug=False)

    ins = {}
    for side in ("s", "d"):
        ins[f"seed_{side}"] = nc.dram_tensor(f"seed_{side}", [D, BL], F32, kind="ExternalInput")
        ins[f"neg_{side}"] = nc.dram_tensor(f"neg_{side}", [D, G1], BF16, kind="ExternalInput")
        ins[f"nn_{side}"] = nc.dram_tensor(f"nn_{side}", [D, G2], F8, kind="ExternalInput")
    for name, shape, dt in (
        ("ident2", [D, 2 * D], F8),
        ("wtopb", [D, D], BF16), ("wbotb", [D, D], BF16),
        ("wtop32", [D, D], F32), ("wbot32", [D, D], F32),
        ("w1t", [D, D], F32), ("w1b", [D, D], F32),
        ("w2m", [D, 64], F32), ("w3m", [64, 8], F32), ("w4m", [8, 2], F32),
    ):
        ins[name] = nc.dram_tensor(name, shape, dt, kind="ExternalInput")
    out_dram = nc.dram_tensor("out", [BL, 2], F32, kind="ExternalOutput")

    with tile.TileContext(nc) as tc, ExitStack() as ctx:
        const = ctx.enter_context(tc.tile_pool(name="const", bufs=1))
        persist = ctx.enter_context(tc.tile_pool(name="persist", bufs=1))
        stream = ctx.enter_context(tc.tile_pool(name="stream", bufs=3))
        work = ctx.enter_context(tc.tile_pool(name="work", bufs=3))
        psA = ctx.enter_context(tc.tile_pool(name="psA", bufs=2, space="PSUM"))
        psB = ctx.enter_context(tc.tile_pool(name="psB", bufs=2, space="PSUM"))
        psM = ctx.enter_context(tc.tile_pool(name="psM", bufs=2, space="PSUM"))

        def load_const(name, shape, dt):
            t = const.tile(shape, dt, tag=name)
            nc.gpsimd.dma_start(t[:], ins[name].ap())
            return t

        # order matters: first consts feed the first stream tile's matmuls
        ident2 = load_const("ident2", [D, 2 * D], F8)
        wtopb = load_const("wtopb", [D, D], BF16)
        wbotb = load_const("wbotb", [D, D], BF16)
        negT, seedT = {}, {}
        for side in ("s", "d"):
            negT[side] = const.tile([D, G1], BF16, tag=f"negT_{side}", name=f"negT_{side}")
            nc.gpsimd.dma_start(negT[side][:], ins[f"neg_{side}"].ap())
            seedT[side] = const.tile([D, BL], F32, tag=f"seedT_{side}", name=f"seedT_{side}")
            nc.gpsimd.dma_start(seedT[side][:], ins[f"seed_{side}"].ap())
        wtop32 = load_const("wtop32", [D, D], F32)
        wbot32 = load_const("wbot32", [D, D], F32)
        w1t = load_const("w1t", [D, D], F32)
        w1b = load_const("w1b", [D, D], F32)
        w2m = load_const("w2m", [D, 64], F32)
        w3m = load_const("w3m", [64, 8], F32)
        w4m = load_const("w4m", [8, 2], F32)

        idv2 = ident2.rearrange("p (j m) -> p j m", j=2)  # [128, 2, 128]
        id1 = ident2[:, 0:D]                              # [128, 128]

        redT, n1 = {}, {}
        for side in ("s", "d"):
            redT[side] = persist.tile([D, G1], BF16, tag=f"redT_{side}", name=f"redT_{side}")
            n1[side] = persist.tile([D, BL], F32, tag=f"n1_{side}", name=f"n1_{side}")

        oT = {}
        PARTS = [(0, 64), (64, 112), (112, BL)]

        def hop1_part(side, pi):
            lo, hi = PARTS[pi]
            w = hi - lo
            ps_o = psM.tile([D, w], F32, tag="ps_m")
            nc.tensor.matmul(ps_o[:], wtop32[:], seedT[side][:, lo:hi], start=True, stop=False)
            nc.tensor.matmul(ps_o[:], wbot32[:], n1[side][:, lo:hi], start=False, stop=True)
            ot = persist.tile([D, w], F32, tag=f"oT_{side}{pi}")
            nc.scalar.activation(ot[:], ps_o[:], AF.Copy)
            oT[side, pi] = ot

        def mlp_part(pi):
            lo, hi = PARTS[pi]
            w = hi - lo
            ps1 = psM.tile([D, w], F32, tag="ps_m")
            nc.tensor.matmul(ps1[:], w1t[:], oT["s", pi][:], start=True, stop=False)
            nc.tensor.matmul(ps1[:], w1b[:], oT["d", pi][:], start=False, stop=True)
            h1 = work.tile([D, w], F32, tag="h1")
            nc.scalar.activation(h1[:], ps1[:], AF.Relu)

            ps2 = psM.tile([64, w], F32, tag="ps_m")
            nc.tensor.matmul(ps2[:], w2m[:], h1[:])
            h2 = work.tile([64, w], F32, tag="h2")
            nc.scalar.activation(h2[:], ps2[:], AF.Relu)

            ps3 = psM.tile([8, w], F32, tag="ps_m")
            nc.tensor.matmul(ps3[:], w3m[:], h2[:])
            h3 = work.tile([8, w], F32, tag="h3")
            nc.scalar.activation(h3[:], ps3[:], AF.Relu)

            ps4 = psM.tile([w, 2], F32, tag="ps_m")
            nc.tensor.matmul(ps4[:], h3[:], w4m[:])
            lg = work.tile([w, 2], F32, tag="lg")
            nc.scalar.activation(lg[:], ps4[:], AF.Copy)

            nm = work.tile([w, 1], F32, tag="nm")
            nc.vector.reduce_max(nm[:], lg[:], axis=AX.X, negate=True)
            ex = work.tile([w, 2], F32, tag="ex")
            se = work.tile([w, 1], F32, tag="se")
            nc.scalar.activation(ex[:], lg[:], AF.Exp, bias=nm[:], accum_out=se[:])
            rc = work.tile([w, 1], F32, tag="rc")
            nc.vector.reciprocal(rc[:], se[:])
            o = work.tile([w, 2], F32, tag="o")
            nc.vector.tensor_scalar_mul(o[:], ex[:], rc[:])
            # SWDGE: a sync-queue store would head-of-line block stream loads
            nc.gpsimd.dma_start(out_dram.ap()[lo:hi], o[:])

        for side in ("s", "d"):
            for t in range(NT):
                xt = stream.tile([D, TC], F8, tag="xt")
                nc.sync.dma_start(xt[:], ins[f"nn_{side}"].ap()[:, t * TC:(t + 1) * TC])
                # phase-major tile: xr[:, k, :] = phase k's 400 group-columns,
                # contiguous. DR matmuls take [p, 2, TG] phase-pair blocks.
                xr = xt.rearrange("p (k g) -> p k g", k=S)
                ps = psA.tile([D, TG], F32, tag="ps_red")
                for i in range(S // 2):
                    nc.tensor.matmul(
                        ps[:], idv2, xr[:, 2 * i:2 * i + 2, :],
                        start=(i == 0), stop=False, perf_mode=DR,
                    )
                nc.tensor.matmul(ps[:], id1, xr[:, S - 1, :], start=False, stop=True)
                rsl = redT[side][:, t * TG:(t + 1) * TG]
                nc.scalar.activation(rsl, ps[:], AF.Copy)

                ps2 = psB.tile([D, TG], F32, tag="ps_ht")
                nc.tensor.matmul(ps2[:], wtopb[:], negT[side][:, t * TG:(t + 1) * TG], start=True, stop=False)
                nc.tensor.matmul(ps2[:], wbotb[:], rsl, start=False, stop=True)
                # hop-1 group-sum straight from PSUM; hT never hits SBUF
                nc.vector.reduce_sum(
                    n1[side][:, t * SPT:(t + 1) * SPT],
                    ps2.rearrange("p (b s) -> p b s", s=S),
                    axis=AX.X,
                )
                if side == "d":
                    if t == 3:
                        for sd in ("s", "d"):
                            hop1_part(sd, 0)
                        mlp_part(0)
                    elif t == 6:
                        for sd in ("s", "d"):
                            hop1_part(sd, 1)
                        mlp_part(1)
        for sd in ("s", "d"):
            hop1_part(sd, 2)
        mlp_part(2)

    nc.compile()
    return nc


_NC_CACHE = None


def _get_program():
    global _NC_CACHE
    if _NC_CACHE is None:
        _NC_CACHE = _build_program()
    return _NC_CACHE


def kernel(src, src_neg, src_neg_neg, dst, dst_neg, dst_neg_neg, w2, W1, W2, W3, W4,
           _trace=False, **trace_kwargs):
    nc = _get_program()

    w2 = np.asarray(w2, np.float32)
    W1 = np.asarray(W1, np.float32)
    wtop = np.ascontiguousarray(w2[:D])
    wbot = np.ascontiguousarray(w2[D:]) / np.float32(S)
    eye = np.eye(D, dtype=np.float32)
    rep = {
        "ident2": np.concatenate([eye, eye], axis=1).astype(NPF8),
        "wtopb": wtop.astype(NPBF), "wbotb": wbot.astype(NPBF),
        "wtop32": wtop, "wbot32": wbot,
        "w1t": np.ascontiguousarray(W1[:D]),
        "w1b": np.ascontiguousarray(W1[D:]),
        "w2m": np.asarray(W2, np.float32),
        "w3m": np.asarray(W3, np.float32),
        "w4m": np.asarray(W4, np.float32),
    }

    def shardT(x, dt, rows):
        # [NCORES*rows, D] -> fp8/bf16/f32, transposed per core -> [NCORES, D, rows]
        return np.ascontiguousarray(
            np.asarray(x).astype(dt).reshape(NCORES, rows, D).transpose(0, 2, 1)
        )

    def shard_nn(x):
        # [NCORES*G2, D] -> fp8, feat-major + phase-major per 400-group tile:
        # out[c, f, t*TC + k*TG + g] = x[c*G2 + (t*TG+g)*S + k, f]
        return np.ascontiguousarray(
            np.asarray(x).astype(NPF8)
            .reshape(NCORES, NT, TG, S, D)
            .transpose(0, 4, 1, 3, 2)
            .reshape(NCORES, D, G2)
        )

    big = {
        "nn_s": shard_nn(src_neg_neg),
        "nn_d": shard_nn(dst_neg_neg),
        "neg_s": shardT(src_neg, NPBF, G1),
        "neg_d": shardT(dst_neg, NPBF, G1),
        "seed_s": shardT(src, np.float32, BL),
        "seed_d": shardT(dst, np.float32, BL),
    }
    in_maps = []
    for c in range(NCORES):
        m = dict(rep)
        for k, v in big.items():
            m[k] = v[c]
        in_maps.append(m)

    res = run_bass_kernel_spmd(
        nc, in_maps, list(range(NCORES)), trace=_trace, **trace_kwargs
    )
    out = np.concatenate([res.results[c]["out"] for c in range(NCORES)], axis=0)
    if _trace:
        return out, res
    return out


# revision 12
# speedup vs baseline: 2.0360x; 1.0591x over previous
"""GraphSAGE supervised forward on 8 Trainium2 NeuronCores.

Full inputs in, full output out. Data-parallel over the B=1024 seed nodes:
128 seeds per core; neighbor rows shard as contiguous row ranges. Tiny
weights replicated.

v6 design — quantize + transpose on host, PE group-sums, algebraic fold:
  - hop-2 neighbors (the 82MB/core f32 stream) are sent as fp8e4m3 in
    feat-major, PHASE-MAJOR-per-tile layout; hop-1 neighbors as bf16
    [128, 3200]; seeds f32. End-to-end max rel err ~2e-3 (gate 2e-2):
    the two mean-over-25 stages attenuate per-element quantization noise.
  - group-sum of 25 phases runs on the PE as accumulating identity
    matmuls: stationary [I;I] fp8 + DoubleRow packs 2 phases per
    column-slot (12 DR + 1 plain matmul per tile, all moving operands
    contiguous blocks), f32 PSUM accumulation.
  - key fold: the hop-1 mean commutes with the aggregator matmul, so
    per-column hidden states are never materialized. Only per-seed sums
    are kept: redS = DVE reduce of the GS PSUM (25 group-cols -> seed),
    negS = DVE group-sum of negT. Then per side
        m1T  = wtop^T negS + wbot^T redS      (25x mean_j h_j, transposed)
        oT   = wtop^T seedT + wbot^T m1T      (hop-1 output)
    with wbot pre-scaled by 1/25 on host. This removes the per-tile hT
    matmuls + copies that serialized the v3-v5 pipelines.
  - hop-1 + 4-layer MLP + softmax (f32) in 4 parts as seed ranges
    complete; sides interleave per tile; ragged tiles (small first/last)
    cut pipeline fill and tail latency.
"""

import sys

for _p in ("/opt/trn_rl_repo", "/root/.axon_site/_ro/trn_rl_repo"):
    if _p not in sys.path:
        sys.path.append(_p)

import numpy as np
import ml_dtypes
from contextlib import ExitStack

import concourse.bass as bass
import concourse.tile as tile
from concourse import bacc, mybir
from concourse.bass_utils import run_bass_kernel_spmd

B, S, D = 1024, 25, 128
NCORES = 8
BL = B // NCORES          # 128 seeds per core
G1 = BL * S               # 3200 hop-1 rows (= hop-2 groups) per core
G2 = G1 * S               # 80000 hop-2 rows per core

# ragged stream tiles (groups per tile, per side); sum = G1
SIZES = [100, 400, 400, 400, 400, 400, 400, 400, 200, 100]
OFFS = np.cumsum([0] + SIZES).tolist()
NTT = len(SIZES)
assert OFFS[-1] == G1 and all(sz % S == 0 for sz in SIZES)

F32 = mybir.dt.float32
BF16 = mybir.dt.bfloat16
F8 = mybir.dt.float8e4
AX = mybir.AxisListType
AF = mybir.ActivationFunctionType
DR = mybir.MatmulPerfMode.DoubleRow

NPF8 = ml_dtypes.float8_e4m3
NPBF = ml_dtypes.bfloat16


def _build_program():
    nc = bacc.Bacc("TRN2", target_bir_lowering=False, debug=False)

    ins = {}
    for side in ("s", "d"):
        ins[f"seed_{side}"] = nc.dram_tensor(f"seed_{side}", [D, BL], F32, kind="ExternalInput")
        ins[f"neg_{side}"] = nc.dram_tensor(f"neg_{side}", [D, G1], BF16, kind="ExternalInput")
        ins[f"nn_{side}"] = nc.dram_tensor(f"nn_{side}", [D, G2], F8, kind="ExternalInput")
    for name, shape, dt in (
        ("ident2", [D, 2 * D], F8),
        ("wtop32", [D, D], F32), ("wbot32", [D, D], F32),
        ("w1t", [D, D], F32), ("w1b", [D, D], F32),
        ("w2m", [D, 64], F32), ("w3m", [64, 8], F32), ("w4m", [8, 2], F32),
    ):
        ins[name] = nc.dram_tensor(name, shape, dt, kind="ExternalInput")
    out_dram = nc.dram_tensor("out", [BL, 2], F32, kind="ExternalOutput")

    with tile.TileContext(nc) as tc, ExitStack() as ctx:
        const = ctx.enter_context(tc.tile_pool(name="const", bufs=1))
        persist = ctx.enter_context(tc.tile_pool(name="persist", bufs=1))
        stream = ctx.enter_context(tc.tile_pool(name="stream", bufs=4))
        work = ctx.enter_context(tc.tile_pool(name="work", bufs=3))
        psA = ctx.enter_context(tc.tile_pool(name="psA", bufs=4, space="PSUM"))
        psM = ctx.enter_context(tc.tile_pool(name="psM", bufs=2, space="PSUM"))

        def load_const(name, shape, dt):
            t = const.tile(shape, dt, tag=name, name=name)
            nc.gpsimd.dma_start(t[:], ins[name].ap())
            return t

        # order matters: ident2 feeds the first stream tile's matmuls
        ident2 = load_const("ident2", [D, 2 * D], F8)
        negT, seedT = {}, {}
        for side in ("s", "d"):
            negT[side] = const.tile([D, G1], BF16, tag=f"negT_{side}", name=f"negT_{side}")
            nc.gpsimd.dma_start(negT[side][:], ins[f"neg_{side}"].ap())
            seedT[side] = const.tile([D, BL], F32, tag=f"seedT_{side}", name=f"seedT_{side}")
            nc.gpsimd.dma_start(seedT[side][:], ins[f"seed_{side}"].ap())
        wtop32 = load_const("wtop32", [D, D], F32)
        wbot32 = load_const("wbot32", [D, D], F32)
        w1t = load_const("w1t", [D, D], F32)
        w1b = load_const("w1b", [D, D], F32)
        w2m = load_const("w2m", [D, 64], F32)
        w3m = load_const("w3m", [64, 8], F32)
        w4m = load_const("w4m", [8, 2], F32)

        idv2 = ident2.rearrange("p (j m) -> p j m", j=2)  # [128, 2, 128]
        id1 = ident2[:, 0:D]                              # [128, 128]

        negS, redS = {}, {}
        for side in ("s", "d"):
            negS[side] = persist.tile([D, BL], F32, tag=f"negS_{side}", name=f"negS_{side}")
            redS[side] = persist.tile([D, BL], F32, tag=f"redS_{side}", name=f"redS_{side}")

        oT = {}
        PARTS = [(0, 52), (52, 100), (100, 124), (124, BL)]

        def hop1_part(side, pi):
            lo, hi = PARTS[pi]
            w = hi - lo
            # m1T = wtop^T negS + wbot^T redS  (= 25x transposed mean_j h_j)
            ps_m = psM.tile([D, w], F32, tag="ps_m")
            nc.tensor.matmul(ps_m[:], wtop32[:], negS[side][:, lo:hi], start=True, stop=False)
            nc.tensor.matmul(ps_m[:], wbot32[:], redS[side][:, lo:hi], start=False, stop=True)
            m1 = work.tile([D, w], F32, tag="m1")
            nc.scalar.activation(m1[:], ps_m[:], AF.Copy)
            # oT = wtop^T seedT + wbot^T m1T
            ps_o = psM.tile([D, w], F32, tag="ps_m")
            nc.tensor.matmul(ps_o[:], wtop32[:], seedT[side][:, lo:hi], start=True, stop=False)
            nc.tensor.matmul(ps_o[:], wbot32[:], m1[:], start=False, stop=True)
            ot = persist.tile([D, w], F32, tag=f"oT_{side}{pi}")
            nc.scalar.activation(ot[:], ps_o[:], AF.Copy)
            oT[side, pi] = ot

        def mlp_part(pi):
            lo, hi = PARTS[pi]
            w = hi - lo
            ps1 = psM.tile([D, w], F32, tag="ps_m")
            nc.tensor.matmul(ps1[:], w1t[:], oT["s", pi][:], start=True, stop=False)
            nc.tensor.matmul(ps1[:], w1b[:], oT["d", pi][:], start=False, stop=True)
            h1 = work.tile([D, w], F32, tag="h1")
            nc.scalar.activation(h1[:], ps1[:], AF.Relu)

            ps2 = psM.tile([64, w], F32, tag="ps_m")
            nc.tensor.matmul(ps2[:], w2m[:], h1[:])
            h2 = work.tile([64, w], F32, tag="h2")
            nc.scalar.activation(h2[:], ps2[:], AF.Relu)

            ps3 = psM.tile([8, w], F32, tag="ps_m")
            nc.tensor.matmul(ps3[:], w3m[:], h2[:])
            h3 = work.tile([8, w], F32, tag="h3")
            nc.scalar.activation(h3[:], ps3[:], AF.Relu)

            ps4 = psM.tile([w, 2], F32, tag="ps_m")
            nc.tensor.matmul(ps4[:], h3[:], w4m[:])
            lg = work.tile([w, 2], F32, tag="lg")
            nc.scalar.activation(lg[:], ps4[:], AF.Copy)

            nm = work.tile([w, 1], F32, tag="nm")
            nc.vector.reduce_max(nm[:], lg[:], axis=AX.X, negate=True)
            ex = work.tile([w, 2], F32, tag="ex")
            se = work.tile([w, 1], F32, tag="se")
            nc.scalar.activation(ex[:], lg[:], AF.Exp, bias=nm[:], accum_out=se[:])
            rc = work.tile([w, 1], F32, tag="rc")
            nc.vector.reciprocal(rc[:], se[:])
            o = work.tile([w, 2], F32, tag="o")
            nc.vector.tensor_scalar_mul(o[:], ex[:], rc[:])
            # SWDGE: a sync-queue store would head-of-line block stream loads
            nc.gpsimd.dma_start(out_dram.ap()[lo:hi], o[:])

        seeds_done = {"s": 0, "d": 0}
        next_part = [0]

        def maybe_parts():
            while next_part[0] < len(PARTS) and min(seeds_done.values()) >= PARTS[next_part[0]][1]:
                for sd in ("s", "d"):
                    hop1_part(sd, next_part[0])
                mlp_part(next_part[0])
                next_part[0] += 1

        def stream_tile(side, t):
            g0, sz = OFFS[t], SIZES[t]
            xt = stream.tile([D, sz * S], F8, tag="xt", name="xt")
            nc.sync.dma_start(xt[:], ins[f"nn_{side}"].ap()[:, g0 * S:(g0 + sz) * S])
            # phase-major tile: xr[:, k, :] = phase k's sz group-columns
            xr = xt.rearrange("p (k g) -> p k g", k=S)
            ps = psA.tile([D, sz], F32, tag="ps_red")
            for i in range(S // 2):
                nc.tensor.matmul(
                    ps[:], idv2, xr[:, 2 * i:2 * i + 2, :],
                    start=(i == 0), stop=False, perf_mode=DR,
                )
            nc.tensor.matmul(ps[:], id1, xr[:, S - 1, :], start=False, stop=True)
            # per-seed sums straight from PSUM (25 group-cols per seed)
            nc.vector.reduce_sum(
                redS[side][:, g0 // S:(g0 + sz) // S],
                ps.rearrange("p (b s) -> p b s", s=S),
                axis=AX.X,
            )
            seeds_done[side] = (g0 + sz) // S

        for t in range(NTT):
            for side in ("s", "d"):
                stream_tile(side, t)
                if t == 0:
                    # per-side group-sum of negT while the pipeline fills
                    nc.vector.reduce_sum(
                        negS[side][:],
                        negT[side].rearrange("p (b s) -> p b s", s=S),
                        axis=AX.X,
                    )
            maybe_parts()

    nc.compile()
    return nc


_NC_CACHE = None


def _get_program():
    global _NC_CACHE
    if _NC_CACHE is None:
        _NC_CACHE = _build_program()
    return _NC_CACHE


def kernel(src, src_neg, src_neg_neg, dst, dst_neg, dst_neg_neg, w2, W1, W2, W3, W4,
           _trace=False, **trace_kwargs):
    nc = _get_program()

    w2 = np.asarray(w2, np.float32)
    W1 = np.asarray(W1, np.float32)
    wtop = np.ascontiguousarray(w2[:D])
    wbot = np.ascontiguousarray(w2[D:]) / np.float32(S)
    eye = np.eye(D, dtype=np.float32)
    rep = {
        "ident2": np.concatenate([eye, eye], axis=1).astype(NPF8),
        "wtop32": wtop, "wbot32": wbot,
        "w1t": np.ascontiguousarray(W1[:D]),
        "w1b": np.ascontiguousarray(W1[D:]),
        "w2m": np.asarray(W2, np.float32),
        "w3m": np.asarray(W3, np.float32),
        "w4m": np.asarray(W4, np.float32),
    }

    def shardT(x, dt, rows):
        # [NCORES*rows, D] -> transposed per core -> [NCORES, D, rows]
        return np.ascontiguousarray(
            np.asarray(x).astype(dt).reshape(NCORES, rows, D).transpose(0, 2, 1)
        )

    def shard_nn(x):
        # [NCORES*G2, D] -> fp8, feat-major + phase-major per ragged tile:
        # out[c, f, OFFS[t]*S + k*SIZES[t] + g] = x[c*G2 + (OFFS[t]+g)*S + k, f]
        x8 = np.asarray(x).astype(NPF8).reshape(NCORES, G1, S, D)
        out = np.empty((NCORES, D, G2), NPF8)
        for t, sz in enumerate(SIZES):
            g0 = OFFS[t]
            blk = x8[:, g0:g0 + sz]                    # [C, sz, S, D]
            out[:, :, g0 * S:(g0 + sz) * S] = (
                blk.transpose(0, 3, 2, 1).reshape(NCORES, D, sz * S)
            )
        return out

    big = {
        "nn_s": shard_nn(src_neg_neg),
        "nn_d": shard_nn(dst_neg_neg),
        "neg_s": shardT(src_neg, NPBF, G1),
        "neg_d": shardT(dst_neg, NPBF, G1),
        "seed_s": shardT(src, np.float32, BL),
        "seed_d": shardT(dst, np.float32, BL),
    }
    in_maps = []
    for c in range(NCORES):
        m = dict(rep)
        for k, v in big.items():
            m[k] = v[c]
        in_maps.append(m)

    res = run_bass_kernel_spmd(
        nc, in_maps, list(range(NCORES)), trace=_trace, **trace_kwargs
    )
    out = np.concatenate([res.results[c]["out"] for c in range(NCORES)], axis=0)
    if _trace:
        return out, res
    return out
